# revision 1
# baseline (speedup 1.0000x reference)
"""Trainium2 Bass kernel for DirectionalConv2D (wind-directed 5x5 Gaussian blur).

Reference math (per pixel):
    theta = arctan2(v, u+1e-8);  c, s = cos(theta), sin(theta)
    w(dx,dy) = exp(-(dx*c + dy*s)^2 / 4.5)        for dx,dy in [-2..2]
    spread   = sum(w * fire[h+dx, w+dy]) / (sum(w) + 1e-8)   (zero padded)
    out      = clip(0.7*spread + 0.3*fire, 0, 1)

Reformulation (no trig, no divide, measured 69.1us / rel err 1.2e-6 on HW):
  * ss = sin^2 = v^2/(u^2+v^2), cs = sin*cos = u*v/(u^2+v^2); the one
    reciprocal is ir2 = Exp(-Ln(r2)) on the Scalar engine (the
    natural_log_exp_and_others ACT table set covers ln/exp/square/identity
    in a single table load).
  * proj^2 = dx^2 + (dy^2-dx^2)*ss + 2*dx*dy*cs is AFFINE in (ss, cs), so
    each of the 12 symmetric pair weights (w(d) = w(-d)) is ONE Exp
    activation (inputs ss, cs, and two mixtures m12/m1m2).
  * wsum(theta) is even and pi/2-symmetric, so 0.7/(wsum+1e-8) is a 3-term
    cos(4k*theta) Fourier series; cos4t comes from one ACT Square of ss,
    cos8t from another; no division anywhere.
  * Sharding: 8 cores = (batch, H-half). Each partition holds 2 output
    rows; fire is staged [128, 6, 516] (2 rows + 2-row halo, W padded 2)
    so all 25 taps are free-dim offsets.
  * All tensor-tensor work on DVE (GpSimd elementwise would contend for
    SBUF ports and slow BOTH engines ~2.2x); squares/exps/series on ACT.
  * Raw bass (this walrus build rejects >1 sync-wait per instruction, so
    the Tile scheduler is unusable): three DMA queues, wind loads
    prioritized ahead of fire chunks (all transfers share the 16 DMA
    sub-engines), per-engine streams with monotone semaphore thresholds,
    final blend/clip/store split in halves to overlap the store.
"""

import sys

if "/opt/trn_rl_repo" not in sys.path:
    sys.path.insert(0, "/opt/trn_rl_repo")

import numpy as np

B, H, W = 4, 512, 512
N_CORES = 8
HS = H // 2
KI = 1.0 / 4.5
C0 = 0.040093331769199714
C1 = 0.0007997721694363273
C2 = -1.6226127085146848e-06

_NC = None


def _build_nc():
    import math

    import concourse.bass as bass
    import concourse.mybir as mybir

    dt = mybir.dt
    AF = mybir.ActivationFunctionType
    OP = mybir.AluOpType
    k = KI
    f32 = dt.float32

    nc = bass.Bass(detect_race_conditions=False)

    f6_d = nc.dram_tensor("fire6", [128, 6, 516], f32, kind="ExternalInput")
    wu_d = nc.dram_tensor("wu", [128, 1024], f32, kind="ExternalInput")
    wv_d = nc.dram_tensor("wv", [128, 1024], f32, kind="ExternalInput")
    out_d = nc.dram_tensor("out", [128, 1024], f32, kind="ExternalOutput")

    def sb(name, shape):
        return nc.alloc_sbuf_tensor(name, shape, f32).ap()

    f6 = sb("f6", [128, 6, 516])
    wu = sb("wu_t", [128, 1024])
    wv = sb("wv_t", [128, 1024])
    u = sb("u", [128, 1024])
    uu = sb("uu", [128, 1024])
    vv = sb("vv", [128, 1024])
    uv = sb("uv", [128, 1024])
    r2 = sb("r2", [128, 1024])
    lnr = sb("lnr", [128, 1024])
    ir2 = sb("ir2", [128, 1024])
    ss = sb("ss", [128, 1024])
    cs = sb("cs", [128, 1024])
    m12 = sb("m12", [128, 1024])
    m1m2 = sb("m1m2", [128, 1024])
    q = sb("q", [128, 1024])
    t8q = sb("t8q", [128, 1024])
    ser = sb("ser", [128, 1024])
    accv = sb("accv", [128, 1024])
    dummy = sb("dummy_t", [128, 1])
    dummy_in = sb("dummy_in", [128, 1])
    # reused slots (writes provably ordered after the prior readers)
    prodv = vv      # vv last read by DVE op3 (ss); first prod write is later
    inv07 = m12     # m12 last read by ACT w12 (A<=20 watermark before write)
    spf = u         # u last read by DVE op1
    sp07 = uv       # uv last read by DVE op4
    opre = lnr      # lnr last read by ACT A5; write is post-A20
    outt = r2       # r2 last read by ACT A4

    pair_order = [
        (0, 1), (0, 2), (1, 0), (1, 1), (1, -1), (1, 2), (1, -2),
        (2, 0), (2, 1), (2, -1), (2, 2), (2, -2),
    ]
    wts = {p: sb(f"w{p[0]}_{p[1]}", [128, 1024]) for p in pair_order}
    pst = {p: sb(f"ps{p[0]}_{p[1]}", [128, 1024]) for p in pair_order}

    espec = {
        (0, 1): ("ss", -k, 0.0),
        (0, 2): ("ss", -4 * k, 0.0),
        (1, 0): ("ss", k, -k),
        (1, 1): ("cs", -2 * k, -k),
        (1, -1): ("cs", 2 * k, -k),
        (1, 2): ("m12", -3 * k, -k),
        (1, -2): ("m1m2", -3 * k, -k),
        (2, 0): ("ss", 4 * k, -4 * k),
        (2, 1): ("m1m2", 3 * k, -4 * k),
        (2, -1): ("m12", 3 * k, -4 * k),
        (2, 2): ("cs", -8 * k, -4 * k),
        (2, -2): ("cs", 8 * k, -4 * k),
    }

    def V(dx, dy, half=None):
        if half is None:
            return f6[:, 2 + dx : 4 + dx, 2 + dy : 514 + dy]
        # one output row (row `half` of the pair) -> 2D [128, 512]
        return f6[:, 2 + dx + half, 2 + dy : 514 + dy]

    def flat3(ap, half=None):
        if half is None:
            return ap.rearrange("p (a b) -> p a b", a=2)
        return ap[:, half * 512 : half * 512 + 512]


    bias_vals = sorted({bi for _, _, bi in espec.values()} | {1e-8, -math.sqrt(8.0) / 2, -math.sqrt(2.0), C0 - C1 - C2})

    # DMA issues live INSIDE the block (a pre-barrier issue makes the
    # engine-barrier drain wait for the whole transfer); bias const memsets
    # are protected by the SB semaphore instead of a barrier
    with (
        nc.semaphore("squ") as SQU,
        nc.semaphore("sqv") as SQV,
        nc.semaphore("fq0") as FQ0,
        nc.semaphore("fq1") as FQ1,
        nc.semaphore("fq2") as FQ2,
        nc.semaphore("sqo") as SQO,
        nc.semaphore("sa") as A,
        nc.semaphore("sv") as Vs,
        nc.semaphore("sb") as SB,
    ):
        for bi_i, val in enumerate(bias_vals):
            if (f32, val) in nc.const_aps.aps:
                continue
            t = nc.alloc_sbuf_tensor(f"constb{bi_i}", [128, 1], f32)
            nc.gpsimd.memset(t.ap(), val)
            nc.const_aps.aps[(f32, val)] = t.ap()
        nc.gpsimd.memset(dummy_in, 0.0).then_inc(SB, 1)

        with nc.Block() as block:

            @block.gpsimd
            def _(gpsimd):
                # start fire chunk1 only after the (critical-path) wind loads
                # finish: all transfers share the 16 DMA sub-engines
                gpsimd.wait_ge(SQV, 32)
                gpsimd.dma_start(f6[:, 1:5:3, :], f6_d[:, 1:5:3, :]).then_inc(FQ1, 16)

            @block.sync
            def _(sync):
                sync.dma_start(wu[0:64, :], wu_d[0:64, :]).then_inc(SQU, 16)
                sync.dma_start(wv[0:64, :], wv_d[0:64, :]).then_inc(SQV, 16)
                sync.dma_start(f6[0:64, 2:4, :], f6_d[0:64, 2:4, :]).then_inc(FQ0, 16)
                sync.dma_start(f6[0:64, 0:6:5, :], f6_d[0:64, 0:6:5, :]).then_inc(FQ2, 16)
                sync.wait_ge(Vs, 6)
                sync.dma_start(out_d[:, 0:512], outt[:, 0:512]).then_inc(SQO, 16)
                sync.wait_ge(Vs, 7)
                sync.dma_start(out_d[:, 512:1024], outt[:, 512:1024]).then_inc(SQO, 16)

            @block.scalar
            def _(scalar):
                a_count = [0]

                def aop(emit):
                    emit().then_inc(A, 1)
                    a_count[0] += 1

                scalar.dma_start(wu[64:128, :], wu_d[64:128, :]).then_inc(SQU, 16)
                scalar.dma_start(wv[64:128, :], wv_d[64:128, :]).then_inc(SQV, 16)
                scalar.dma_start(f6[64:128, 2:4, :], f6_d[64:128, 2:4, :]).then_inc(FQ0, 16)
                scalar.dma_start(f6[64:128, 0:6:5, :], f6_d[64:128, 0:6:5, :]).then_inc(FQ2, 16)
                scalar.wait_ge(SB, 1)
                # dummy activation first: walrus places the ACT table load
                # before it, off the wu-wait critical path
                aop(lambda: scalar.activation(dummy, dummy_in, AF.Exp))             # A1 (dummy)
                scalar.wait_ge(SQU, 32)
                aop(lambda: scalar.activation(u, wu, AF.Identity, bias=1e-8))       # A2
                aop(lambda: scalar.activation(uu, wu, AF.Square, bias=1e-8))        # A2
                scalar.wait_ge(SQV, 32)
                aop(lambda: scalar.activation(vv, wv, AF.Square))
                scalar.wait_ge(Vs, 1)
                aop(lambda: scalar.activation(lnr, r2, AF.Ln))                      # A4
                scalar.wait_ge(A, 5)  # ACT pipeline RAW on lnr
                aop(lambda: scalar.activation(ir2, lnr, AF.Exp, scale=-1.0))        # A5
                # exps in MAC consumption order
                srcmap = {"ss": (ss, 2), "cs": (cs, 3), "m12": (m12, 4), "m1m2": (m1m2, 5)}  # Vs ticks
                waited = [0]

                def exp_of(p):
                    srcname, sc, bi = espec[p]
                    src, need = srcmap[srcname]
                    if need > waited[0]:
                        scalar.wait_ge(Vs, need)
                        waited[0] = need
                    aop(lambda: scalar.activation(wts[p], src, AF.Exp, bias=bi, scale=sc))

                for p in pair_order[:9]:       # A6..A14 (w01..w21)
                    exp_of(p)
                exp_of((2, -1))                # A15
                s8 = math.sqrt(8.0)
                aop(lambda: scalar.activation(q, ss, AF.Square, bias=-s8 / 2, scale=s8))   # A16
                exp_of((2, 2))                 # A17
                s2_ = math.sqrt(2.0)
                aop(lambda: scalar.activation(t8q, q, AF.Square, bias=-s2_, scale=s2_))    # A18
                exp_of((2, -2))                # A19
                aop(lambda: scalar.activation(ser, q, AF.Identity, bias=C0 - C1 - C2, scale=C1))  # A20
                assert a_count[0] == 21

            @block.vector
            def _(vector):
                vector.wait_ge(SQV, 32)
                vector.wait_ge(A, 2)
                vector.tensor_tensor(uv, u, wv, OP.mult)                      # op1
                vector.wait_ge(A, 4)
                vector.tensor_tensor(r2, uu, vv, OP.add).then_inc(Vs, 1)      # V1
                # two pairsums while ACT computes ln/exp for ir2
                vector.wait_ge(FQ0, 32)
                p0, p1 = pair_order[0], pair_order[1]
                vector.tensor_tensor(flat3(pst[p0]), V(*p0), V(-p0[0], -p0[1]), OP.add)
                vector.tensor_tensor(flat3(pst[p1]), V(*p1), V(-p1[0], -p1[1]), OP.add)
                vector.wait_ge(A, 6)
                vector.tensor_tensor(ss, vv, ir2, OP.mult).then_inc(Vs, 1)    # V2
                vector.tensor_tensor(cs, uv, ir2, OP.mult).then_inc(Vs, 1)    # V3
                vector.scalar_tensor_tensor(m12, cs, 4.0 / 3.0, ss, OP.mult, OP.add).then_inc(Vs, 1)    # V4
                vector.scalar_tensor_tensor(m1m2, cs, -4.0 / 3.0, ss, OP.mult, OP.add).then_inc(Vs, 1)  # V5
                # remaining pairsums
                vector.wait_ge(FQ1, 16)
                for i, p in enumerate(pair_order[2:7], start=2):
                    vector.tensor_tensor(flat3(pst[p]), V(*p), V(-p[0], -p[1]), OP.add)
                vector.wait_ge(FQ2, 32)
                for p in pair_order[7:]:
                    vector.tensor_tensor(flat3(pst[p]), V(*p), V(-p[0], -p[1]), OP.add)
                # MAC
                athr = {p: 6 + i + 1 for i, p in enumerate(pair_order[:9])}
                athr[(2, -1)] = 16
                athr[(2, 2)] = 18
                athr[(2, -2)] = 20
                awaited = [6]
                for i, p in enumerate(pair_order):
                    if athr[p] > awaited[0]:
                        vector.wait_ge(A, athr[p])
                        awaited[0] = athr[p]
                    tgt = accv if i == 0 else prodv
                    vector.tensor_tensor(tgt, wts[p], pst[p], OP.mult)
                    if i > 0:
                        vector.tensor_tensor(accv, accv, prodv, OP.add)
                vector.wait_ge(A, 21)
                vector.scalar_tensor_tensor(inv07, t8q, C2, ser, OP.mult, OP.add)
                # final blend/clip in halves, store overlaps
                for h in (0, 1):
                    hs = slice(h * 512, h * 512 + 512)
                    vector.tensor_tensor(flat3(spf, h), flat3(accv, h), V(0, 0, h), OP.add)
                    vector.tensor_tensor(sp07[:, hs], spf[:, hs], inv07[:, hs], OP.mult)
                    vector.scalar_tensor_tensor(
                        flat3(opre, h), V(0, 0, h), 0.3, flat3(sp07, h), OP.mult, OP.add
                    )
                    vector.tensor_scalar(
                        out=outt[:, hs], in0=opre[:, hs], scalar1=0.0, scalar2=1.0,
                        op0=OP.max, op1=OP.min,
                    ).then_inc(Vs, 1)   # V6, V7

    return nc


def _get_nc():
    global _NC
    if _NC is None:
        _NC = _build_nc()
    return _NC


def _make_in_maps(fire_map, wind_u, wind_v):
    from numpy.lib.stride_tricks import sliding_window_view

    in_maps = []
    for b in range(B):
        fp = np.pad(np.asarray(fire_map[b, 0], np.float32), ((2, 2), (2, 2)))
        for t in range(2):
            shard = fp[t * HS : t * HS + HS + 4]
            f6 = np.ascontiguousarray(
                sliding_window_view(shard, (6, 516))[::2, 0], dtype=np.float32
            )
            wu = np.ascontiguousarray(
                np.asarray(wind_u[b, 0, t * HS : (t + 1) * HS], np.float32).reshape(128, 1024)
            )
            wv = np.ascontiguousarray(
                np.asarray(wind_v[b, 0, t * HS : (t + 1) * HS], np.float32).reshape(128, 1024)
            )
            in_maps.append({"fire6": f6, "wu": wu, "wv": wv})
    return in_maps


def _gather(results):
    out = np.empty((B, 1, H, W), np.float32)
    for ci, r in enumerate(results):
        b, t = divmod(ci, 2)
        out[b, 0, t * HS : (t + 1) * HS] = r["out"].reshape(HS, W)
    return out


def _run(fire_map, wind_u, wind_v, trace=False):
    from concourse.bass_utils import run_bass_kernel_spmd

    in_maps = _make_in_maps(fire_map, wind_u, wind_v)
    res = run_bass_kernel_spmd(_get_nc(), in_maps, list(range(N_CORES)), trace=trace)
    return _gather(res.results), res


def kernel(fire_map, wind_u, wind_v):
    out, _ = _run(fire_map, wind_u, wind_v, trace=False)
    return out



# revision 6
# speedup vs baseline: 1.6940x; 1.6940x over previous
"""Trainium2 Bass kernel for DirectionalConv2D (wind-directed 5x5 Gaussian blur).

Reference math (per pixel):
    theta = arctan2(v, u+1e-8);  c, s = cos(theta), sin(theta)
    w(dx,dy) = exp(-(dx*c + dy*s)^2 / 4.5)        for dx,dy in [-2..2]
    spread   = sum(w * fire[h+dx, w+dy]) / (sum(w) + 1e-8)   (zero padded)
    out      = clip(0.7*spread + 0.3*fire, 0, 1)

v2 reformulation (fp16 hot path; rel-err budget 2e-2, this sits ~1e-3):
  * ss = sin^2 = v^2/(u^2+v^2), cs' = (4/3)*u*v/(u^2+v^2); the reciprocal is
    ir2 = Exp(-Ln(r2)), and ir243 = Exp(-Ln(r2)+ln(4/3)) folds the 4/3 so
    m12 = ss+cs', m1m2 = ss-cs' are plain fp16 TTs (STT has no DVE 2x mode).
  * proj^2 is affine in (ss, cs): each of the 12 symmetric pair weights is
    ONE Exp activation reading ss/cs'/m12/m1m2, written in fp16.
  * 0.7/(wsum+1e-8) is a 3-term cos(4k*theta) Fourier series (no divide).
  * fp16 everywhere bandwidth-bound on DVE: fire staged fp16 (host cast),
    pairsums/MAC/finals all fp16 TENSOR_TENSOR -> DVE 2x mode (~770ns vs
    1465ns per [128,1024] op).  Wind path stays fp32 (ir2 range + accuracy).
  * Wind DMA split across 4 queues (sync/scalar/vector/tensor) so uuvv can
    start ~5us earlier than the baseline's 2-queue load; fire fp16 in 3
    chunks on the gpsimd queue (center rows first -> dx=0 pairsums start
    while wind still loading).
  * uu+vv fused into ONE [128,2048] ACT Square over the packed wind buffer.
  * Raw bass (walrus rejects >1 sync-wait per instruction): per-engine
    streams with monotone semaphore thresholds; final blend/clip/store
    split in halves to overlap the store.
"""

import sys

if "/opt/trn_rl_repo" not in sys.path:
    sys.path.insert(0, "/opt/trn_rl_repo")

import numpy as np

B, H, W = 4, 512, 512
N_CORES = 8
HS = H // 2
KI = 1.0 / 4.5
C0 = 0.040093331769199714
C1 = 0.0007997721694363273
C2 = -1.6226127085146848e-06

_NC = None


def _build_nc():
    import math

    import concourse.bass as bass
    import concourse.mybir as mybir

    dt = mybir.dt
    AF = mybir.ActivationFunctionType
    OP = mybir.AluOpType
    k = KI
    f32 = dt.float32
    f16 = dt.float16

    nc = bass.Bass(detect_race_conditions=False)

    f6_d = nc.dram_tensor("fire6", [128, 6, 516], f16, kind="ExternalInput")
    w2_d = nc.dram_tensor("w2", [128, 2048], f32, kind="ExternalInput")  # wu|wv
    out_d = nc.dram_tensor("out", [128, 1024], f16, kind="ExternalOutput")

    def sb(name, shape, dtype=f32):
        return nc.alloc_sbuf_tensor(name, shape, dtype).ap()

    f6 = sb("f6", [128, 6, 516], f16)
    w2 = sb("w2_t", [128, 2048])          # cols 0:1024 wu, 1024:2048 wv
    uuvv = sb("uuvv", [128, 2048])
    uv = sb("uv", [128, 1024])
    r2 = sb("r2", [128, 1024])
    lnr = sb("lnr", [128, 1024])
    ir2 = sb("ir2", [128, 1024])
    ir243 = sb("ir243", [128, 1024])
    ss = sb("ss", [128, 1024], f16)
    cs = sb("cs", [128, 1024], f16)       # holds (4/3)*sin*cos
    m12 = sb("m12", [128, 1024], f16)
    m1m2 = sb("m1m2", [128, 1024], f16)
    q = sb("q", [128, 1024], f16)
    t8q = sb("t8q", [128, 1024], f16)
    ser = sb("ser", [128, 1024], f16)
    f03 = sb("f03", [128, 1024], f16)
    inv07 = sb("inv07", [128, 1024], f16)
    accv = sb("accv", [128, 1024], f16)
    prodv = sb("prodv", [128, 1024], f16)
    spf = sb("spf", [128, 1024], f16)
    sp07 = sb("sp07", [128, 1024], f16)
    opre = sb("opre", [128, 1024], f16)
    outt = sb("outt", [128, 1024], f16)
    dummy = sb("dummy_t", [128, 1])
    dummy_in = sb("dummy_in", [128, 1])

    # MAC order groups by exp source so ACT can stream them just-in-time:
    # 4 ss-exps, 2 cs-exps, m12/m1m2 exps, then the remaining cs-exps.
    pair_order = [
        (0, 1), (0, 2), (1, 0), (2, 0), (1, 1), (1, -1),
        (1, 2), (1, -2), (2, -1), (2, 1), (2, 2), (2, -2),
    ]
    wts = {p: sb(f"w{p[0]}_{p[1]}", [128, 1024], f16) for p in pair_order}
    pst = {p: sb(f"ps{p[0]}_{p[1]}", [128, 1024], f16) for p in pair_order}

    # exp spec: (source, scale, bias) with cs already carrying the 4/3.
    espec = {
        (0, 1): ("ss", -k, 0.0),
        (0, 2): ("ss", -4 * k, 0.0),
        (1, 0): ("ss", k, -k),
        (2, 0): ("ss", 4 * k, -4 * k),
        (1, 1): ("cs", -1.5 * k, -k),
        (1, -1): ("cs", 1.5 * k, -k),
        (1, 2): ("m12", -3 * k, -k),
        (1, -2): ("m1m2", -3 * k, -k),
        (2, -1): ("m12", 3 * k, -4 * k),
        (2, 1): ("m1m2", 3 * k, -4 * k),
        (2, 2): ("cs", -6 * k, -4 * k),
        (2, -2): ("cs", 6 * k, -4 * k),
    }

    def V(dx, dy, half=None):
        if half is None:
            return f6[:, 2 + dx : 4 + dx, 2 + dy : 514 + dy]
        return f6[:, 2 + dx + half, 2 + dy : 514 + dy]

    def flat3(ap, half=None):
        if half is None:
            return ap.rearrange("p (a b) -> p a b", a=2)
        return ap[:, half * 512 : half * 512 + 512]

    s8 = math.sqrt(8.0)
    s2_ = math.sqrt(2.0)
    bias_vals = sorted(
        {bi for _, _, bi in espec.values()}
        | {1e-8, -s8 / 2, -s2_, C0 - C1 - C2, math.log(4.0 / 3.0)}
    )

    with (
        nc.semaphore("wnd") as WND,
        nc.semaphore("fc") as FC,
        nc.semaphore("f14") as F14,
        nc.semaphore("f05") as F05,
        nc.semaphore("sqo") as SQO,
        nc.semaphore("sa") as A,
        nc.semaphore("sv") as Vs,
        nc.semaphore("sb") as SB,
    ):
        for bi_i, val in enumerate(bias_vals):
            if (f32, val) in nc.const_aps.aps:
                continue
            t = nc.alloc_sbuf_tensor(f"constb{bi_i}", [128, 1], f32)
            nc.gpsimd.memset(t.ap(), val)
            nc.const_aps.aps[(f32, val)] = t.ap()
        nc.gpsimd.memset(dummy_in, 0.0).then_inc(SB, 1)

        # exp ACT tick per pair (A counts every ACT op, in issue order):
        # 1 dummy, 2 uuvv, 3 lnr, 4 ir2, 5 ir243,
        # 6 w01, 7 w02, 8 w10, 9 w20, 10 w11, 11 w1m1, 12 q, 13 w12,
        # 14 w1m2, 15 t8q, 16 w2m1, 17 w21, 18 ser, 19 w22, 20 w2m2, 21 f03
        exp_tick = {
            (0, 1): 6, (0, 2): 7, (1, 0): 8, (2, 0): 9, (1, 1): 10,
            (1, -1): 11, (1, 2): 13, (1, -2): 14, (2, -1): 16, (2, 1): 17,
            (2, 2): 19, (2, -2): 20,
        }

        with nc.Block() as block:

            @block.sync
            def _(sync):
                sync.dma_start(w2[0:64, 0:1024], w2_d[0:64, 0:1024]).then_inc(WND, 16)
                sync.dma_start(w2[0:64, 1024:2048], w2_d[0:64, 1024:2048]).then_inc(WND, 16)
                sync.wait_ge(Vs, 6)
                sync.dma_start(out_d[:, 0:512], outt[:, 0:512]).then_inc(SQO, 16)
                sync.wait_ge(Vs, 7)
                sync.dma_start(out_d[:, 512:1024], outt[:, 512:1024]).then_inc(SQO, 16)

            @block.gpsimd
            def _(gpsimd):
                gpsimd.dma_start(f6[:, 2:4, :], f6_d[:, 2:4, :]).then_inc(FC, 16)
                gpsimd.dma_start(f6[:, 1:5:3, :], f6_d[:, 1:5:3, :]).then_inc(F14, 16)
                gpsimd.dma_start(f6[:, 0:6:5, :], f6_d[:, 0:6:5, :]).then_inc(F05, 16)

            @block.scalar
            def _(scalar):
                a_count = [0]

                def aop(emit):
                    emit().then_inc(A, 1)
                    a_count[0] += 1

                scalar.dma_start(w2[64:128, 0:1024], w2_d[64:128, 0:1024]).then_inc(WND, 16)
                scalar.dma_start(w2[64:128, 1024:2048], w2_d[64:128, 1024:2048]).then_inc(WND, 16)
                scalar.wait_ge(SB, 1)
                # dummy first: walrus places the ACT table load before it
                aop(lambda: scalar.activation(dummy, dummy_in, AF.Exp))          # A1
                scalar.wait_ge(WND, 64)
                aop(lambda: scalar.activation(uuvv, w2, AF.Square, bias=1e-8))   # A2
                scalar.wait_ge(Vs, 1)
                aop(lambda: scalar.activation(lnr, r2, AF.Ln))                   # A3
                scalar.wait_ge(A, 3)  # ACT pipeline RAW on lnr
                aop(lambda: scalar.activation(ir2, lnr, AF.Exp, scale=-1.0))     # A4
                aop(lambda: scalar.activation(ir243, lnr, AF.Exp, scale=-1.0,
                                              bias=math.log(4.0 / 3.0)))         # A5

                srcmap = {"ss": (ss, 2), "cs": (cs, 3), "m12": (m12, 4), "m1m2": (m1m2, 5)}
                waited = [1]

                def exp_of(p):
                    srcname, sc, bi = espec[p]
                    src, need = srcmap[srcname]
                    if need > waited[0]:
                        scalar.wait_ge(Vs, need)
                        waited[0] = need
                    aop(lambda: scalar.activation(wts[p], src, AF.Exp, bias=bi, scale=sc))

                for p in pair_order[:6]:            # A6..A11 (w01..w1m1)
                    exp_of(p)
                aop(lambda: scalar.activation(q, ss, AF.Square, bias=-s8 / 2, scale=s8))   # A12
                exp_of((1, 2))                       # A13
                exp_of((1, -2))                      # A14
                scalar.wait_ge(A, 12)  # ACT pipeline RAW on q
                aop(lambda: scalar.activation(t8q, q, AF.Square, bias=-s2_, scale=s2_))    # A15
                exp_of((2, -1))                      # A16
                exp_of((2, 1))                       # A17
                aop(lambda: scalar.activation(ser, q, AF.Identity, bias=C0 - C1 - C2, scale=C1))  # A18
                exp_of((2, 2))                       # A19
                exp_of((2, -2))                      # A20
                scalar.wait_ge(FC, 16)
                aop(lambda: scalar.activation(flat3(f03), V(0, 0), AF.Copy, scale=0.3))  # A21
                assert a_count[0] == 21

            @block.vector
            def _(vector):
                # dx=0 pairsums as soon as fire center rows land
                vector.wait_ge(FC, 16)
                for p in [(0, 1), (0, 2)]:
                    vector.tensor_tensor(flat3(pst[p]), V(*p), V(-p[0], -p[1]), OP.add)
                vector.wait_ge(WND, 64)
                vector.tensor_tensor(uv, w2[:, 0:1024], w2[:, 1024:2048], OP.mult)
                vector.wait_ge(A, 2)
                vector.tensor_tensor(r2, uuvv[:, 0:1024], uuvv[:, 1024:2048], OP.add).then_inc(Vs, 1)
                # dx=1 pairsums while ACT does ln/exp
                vector.wait_ge(F14, 16)
                for p in [(1, 0), (1, 1), (1, -1), (1, 2), (1, -2)]:
                    vector.tensor_tensor(flat3(pst[p]), V(*p), V(-p[0], -p[1]), OP.add)
                vector.wait_ge(A, 4)
                vector.tensor_tensor(ss, uuvv[:, 1024:2048], ir2, OP.mult).then_inc(Vs, 1)
                vector.wait_ge(A, 5)
                vector.tensor_tensor(cs, uv, ir243, OP.mult).then_inc(Vs, 1)
                vector.tensor_tensor(m12, ss, cs, OP.add).then_inc(Vs, 1)
                vector.tensor_tensor(m1m2, ss, cs, OP.subtract).then_inc(Vs, 1)
                # dx=2 pairsums
                vector.wait_ge(F05, 16)
                for p in [(2, 0), (2, 1), (2, -1), (2, 2), (2, -2)]:
                    vector.tensor_tensor(flat3(pst[p]), V(*p), V(-p[0], -p[1]), OP.add)
                # MAC
                awaited = [5]
                for i, p in enumerate(pair_order):
                    if exp_tick[p] > awaited[0]:
                        vector.wait_ge(A, exp_tick[p])
                        awaited[0] = exp_tick[p]
                    tgt = accv if i == 0 else prodv
                    vector.tensor_tensor(tgt, wts[p], pst[p], OP.mult)
                    if i > 0:
                        vector.tensor_tensor(accv, accv, prodv, OP.add)
                vector.wait_ge(A, 18)
                vector.scalar_tensor_tensor(inv07, t8q, C2, ser, OP.mult, OP.add)
                vector.wait_ge(A, 21)
                for h in (0, 1):
                    hs = slice(h * 512, h * 512 + 512)
                    vector.tensor_tensor(flat3(spf, h), flat3(accv, h), V(0, 0, h), OP.add)
                    vector.tensor_tensor(sp07[:, hs], spf[:, hs], inv07[:, hs], OP.mult)
                    vector.tensor_tensor(opre[:, hs], sp07[:, hs], f03[:, hs], OP.add)
                    vector.tensor_scalar(
                        out=outt[:, hs], in0=opre[:, hs], scalar1=0.0, scalar2=1.0,
                        op0=OP.max, op1=OP.min,
                    ).then_inc(Vs, 1)   # Vs 6, 7

    return nc


def _get_nc():
    global _NC
    if _NC is None:
        _NC = _build_nc()
    return _NC


def _make_in_maps(fire_map, wind_u, wind_v):
    from numpy.lib.stride_tricks import sliding_window_view

    in_maps = []
    for b in range(B):
        fp = np.pad(
            np.asarray(fire_map[b, 0], np.float32), ((2, 2), (2, 2))
        ).astype(np.float16)
        for t in range(2):
            shard = fp[t * HS : t * HS + HS + 4]
            f6 = np.ascontiguousarray(
                sliding_window_view(shard, (6, 516))[::2, 0], dtype=np.float16
            )
            w2 = np.empty((128, 2048), np.float32)
            w2[:, 0:1024] = np.asarray(
                wind_u[b, 0, t * HS : (t + 1) * HS], np.float32
            ).reshape(128, 1024)
            w2[:, 1024:2048] = np.asarray(
                wind_v[b, 0, t * HS : (t + 1) * HS], np.float32
            ).reshape(128, 1024)
            in_maps.append({"fire6": f6, "w2": w2})
    return in_maps


def _gather(results):
    out = np.empty((B, 1, H, W), np.float32)
    for ci, r in enumerate(results):
        b, t = divmod(ci, 2)
        out[b, 0, t * HS : (t + 1) * HS] = r["out"].astype(np.float32).reshape(HS, W)
    return out


def _run(fire_map, wind_u, wind_v, trace=False):
    from concourse.bass_utils import run_bass_kernel_spmd

    in_maps = _make_in_maps(fire_map, wind_u, wind_v)
    res = run_bass_kernel_spmd(_get_nc(), in_maps, list(range(N_CORES)), trace=trace)
    return _gather(res.results), res


def kernel(fire_map, wind_u, wind_v):
    out, _ = _run(fire_map, wind_u, wind_v, trace=False)
    return out


# revision 7
# speedup vs baseline: 1.8580x; 1.0968x over previous
"""Trainium2 Bass kernel for DirectionalConv2D (wind-directed 5x5 Gaussian blur).

Reference math (per pixel):
    theta = arctan2(v, u+1e-8);  c, s = cos(theta), sin(theta)
    w(dx,dy) = exp(-(dx*c + dy*s)^2 / 4.5)        for dx,dy in [-2..2]
    spread   = sum(w * fire[h+dx, w+dy]) / (sum(w) + 1e-8)   (zero padded)
    out      = clip(0.7*spread + 0.3*fire, 0, 1)

v3 reformulation (16-bit everywhere it pays; rel-err budget 2e-2, sim 5.5e-4):
  * ss = sin^2 = v^2/r2, cs = (4/3)*u*v/r2 via ir2 = Exp(-Ln(r2)) and
    ir243 = Exp(-Ln(r2)+ln(4/3)); folding 4/3 into cs makes m12 = ss+cs,
    m1m2 = ss-cs plain TTs.  Each of the 12 symmetric pair weights is ONE
    Exp activation (args affine in ss/cs/m12/m1m2), written fp16.
  * 0.7/(wsum+1e-8) is a 3-term cos(4k theta) Fourier series; the final
    blend is inv07 = t8c2 + ser (both ACT outputs) so no STT anywhere hot.
  * DVE 2x mode (2-byte dtypes, packed, SBUF) halves TENSOR_TENSOR time:
    wind/uuvv/ir2/uv in bf16 (range!), fire/weights/MAC/finals in fp16.
    Only lnr stays fp32 (|lnr| up to ~35 would lose Exp accuracy in 16b).
  * DMA: wind bf16 (512KB) ahead of the strided fire rows on the two HW
    queues; fire center rows alone on the gpsimd queue so dx=0 pairsums
    start first; const memsets pushed after the DMA issues.  Transfers on
    one queue complete nearly together (descriptor interleave), so each
    queue carries its gating transfer first.
  * dx=2 pairsums live between MAC pairs 7 and 8 (their fire rows arrive
    last; the MAC's first 7 pairs only need dx<=1 rows).
  * Raw bass (walrus rejects >1 sync-wait per instruction): per-engine
    streams with monotone semaphore thresholds; final blend/clip/store in
    halves to overlap the store.
"""

import sys

if "/opt/trn_rl_repo" not in sys.path:
    sys.path.insert(0, "/opt/trn_rl_repo")

import numpy as np

B, H, W = 4, 512, 512
N_CORES = 8
HS = H // 2
KI = 1.0 / 4.5
C0 = 0.040093331769199714
C1 = 0.0007997721694363273
C2 = -1.6226127085146848e-06

_NC = None


def _build_nc():
    import math

    import concourse.bass as bass
    import concourse.mybir as mybir

    dt = mybir.dt
    AF = mybir.ActivationFunctionType
    OP = mybir.AluOpType
    k = KI
    f32 = dt.float32
    f16 = dt.float16
    bf16 = dt.bfloat16

    nc = bass.Bass(detect_race_conditions=False)

    f6_d = nc.dram_tensor("fire6", [128, 6, 516], f16, kind="ExternalInput")
    w2_d = nc.dram_tensor("w2", [128, 2048], bf16, kind="ExternalInput")  # wu|wv
    out_d = nc.dram_tensor("out", [128, 1024], f16, kind="ExternalOutput")

    def sb(name, shape, dtype=f32):
        return nc.alloc_sbuf_tensor(name, shape, dtype).ap()

    f6 = sb("f6", [128, 6, 516], f16)
    w2 = sb("w2_t", [128, 2048], bf16)    # cols 0:1024 wu, 1024:2048 wv
    uuvv = sb("uuvv", [128, 2048], bf16)
    uv = sb("uv", [128, 1024], bf16)
    r2 = sb("r2", [128, 1024], bf16)
    lnr = sb("lnr", [128, 1024])
    ir2 = sb("ir2", [128, 1024], bf16)
    ir243 = sb("ir243", [128, 1024], bf16)
    ss = sb("ss", [128, 1024], f16)
    cs = sb("cs", [128, 1024], f16)       # holds (4/3)*sin*cos
    m12 = sb("m12", [128, 1024], f16)
    m1m2 = sb("m1m2", [128, 1024], f16)
    q = sb("q", [128, 1024], f16)
    t8q = sb("t8q", [128, 1024], f16)
    t8c2 = sb("t8c2", [128, 1024], f16)
    ser = sb("ser", [128, 1024], f16)
    f03 = sb("f03", [128, 1024], f16)
    inv07 = sb("inv07", [128, 1024], f16)
    accv = sb("accv", [128, 1024], f16)
    prodv = sb("prodv", [128, 1024], f16)
    spf = sb("spf", [128, 1024], f16)
    sp07 = sb("sp07", [128, 1024], f16)
    opre = sb("opre", [128, 1024], f16)
    outt = sb("outt", [128, 1024], f16)
    dummy = sb("dummy_t", [128, 1])
    dummy_in = sb("dummy_in", [128, 1])

    # dx=2 pairs last: their fire rows (0,5) arrive last; MAC pairs 1-7 need
    # only rows 1..4.
    pair_order = [
        (0, 1), (0, 2), (1, 0), (1, 1), (1, -1), (1, 2), (1, -2),
        (2, 0), (2, -1), (2, 1), (2, 2), (2, -2),
    ]
    wts = {p: sb(f"w{p[0]}_{p[1]}", [128, 1024], f16) for p in pair_order}
    pst = {p: sb(f"ps{p[0]}_{p[1]}", [128, 1024], f16) for p in pair_order}

    # exp spec: (source, scale, bias) with cs already carrying the 4/3.
    espec = {
        (0, 1): ("ss", -k, 0.0),
        (0, 2): ("ss", -4 * k, 0.0),
        (1, 0): ("ss", k, -k),
        (2, 0): ("ss", 4 * k, -4 * k),
        (1, 1): ("cs", -1.5 * k, -k),
        (1, -1): ("cs", 1.5 * k, -k),
        (1, 2): ("m12", -3 * k, -k),
        (1, -2): ("m1m2", -3 * k, -k),
        (2, -1): ("m12", 3 * k, -4 * k),
        (2, 1): ("m1m2", 3 * k, -4 * k),
        (2, 2): ("cs", -6 * k, -4 * k),
        (2, -2): ("cs", 6 * k, -4 * k),
    }

    def V(dx, dy, half=None):
        if half is None:
            return f6[:, 2 + dx : 4 + dx, 2 + dy : 514 + dy]
        return f6[:, 2 + dx + half, 2 + dy : 514 + dy]

    def flat3(ap, half=None):
        if half is None:
            return ap.rearrange("p (a b) -> p a b", a=2)
        return ap[:, half * 512 : half * 512 + 512]

    s8 = math.sqrt(8.0)
    s2_ = math.sqrt(2.0)
    bias_vals = sorted(
        {bi for _, _, bi in espec.values()}
        | {1e-8, -s8 / 2, -s2_, C0 - C1 - C2, math.log(4.0 / 3.0)}
    )

    with (
        nc.semaphore("wnd") as WND,
        nc.semaphore("fc") as FC,
        nc.semaphore("f14") as F14,
        nc.semaphore("f05") as F05,
        nc.semaphore("sqo") as SQO,
        nc.semaphore("sa") as A,
        nc.semaphore("sv") as Vs,
        nc.semaphore("sb") as SB,
    ):
        # ACT tick per op, in issue order:
        # 1 dummy, 2 uuvv, 3 lnr, 4 ir2, 5 ir243,
        # 6 w01, 7 w02, 8 w10, 9 w11, 10 w1m1, 11 q, 12 w12, 13 w1m2,
        # 14 t8q, 15 w20, 16 w2m1, 17 t8c2, 18 ser, 19 w21, 20 w22,
        # 21 w2m2, 22 f03
        exp_tick = {
            (0, 1): 6, (0, 2): 7, (1, 0): 8, (1, 1): 9, (1, -1): 10,
            (1, 2): 12, (1, -2): 13, (2, 0): 15, (2, -1): 16, (2, 1): 19,
            (2, 2): 20, (2, -2): 21,
        }

        with nc.Block() as block:

            @block.sync
            def _(sync):
                sync.dma_start(w2[0:64, 0:1024], w2_d[0:64, 0:1024]).then_inc(WND, 16)
                sync.dma_start(w2[0:64, 1024:2048], w2_d[0:64, 1024:2048]).then_inc(WND, 16)
                sync.dma_start(f6[0:64, 1:5:3, :], f6_d[0:64, 1:5:3, :]).then_inc(F14, 16)
                sync.dma_start(f6[0:64, 0:6:5, :], f6_d[0:64, 0:6:5, :]).then_inc(F05, 16)
                sync.wait_ge(Vs, 6)
                sync.dma_start(out_d[:, 0:512], outt[:, 0:512]).then_inc(SQO, 16)
                sync.wait_ge(Vs, 7)
                sync.dma_start(out_d[:, 512:1024], outt[:, 512:1024]).then_inc(SQO, 16)

            @block.gpsimd
            def _(gpsimd):
                gpsimd.dma_start(f6[:, 2:4, :], f6_d[:, 2:4, :]).then_inc(FC, 16)
                for bi_i, val in enumerate(bias_vals):
                    if (f32, val) in nc.const_aps.aps:
                        continue
                    t = nc.alloc_sbuf_tensor(f"constb{bi_i}", [128, 1], f32)
                    gpsimd.memset(t.ap(), val)
                    nc.const_aps.aps[(f32, val)] = t.ap()
                gpsimd.memset(dummy_in, 0.0).then_inc(SB, 1)

            @block.scalar
            def _(scalar):
                a_count = [0]

                def aop(emit):
                    emit().then_inc(A, 1)
                    a_count[0] += 1

                scalar.dma_start(w2[64:128, 0:1024], w2_d[64:128, 0:1024]).then_inc(WND, 16)
                scalar.dma_start(w2[64:128, 1024:2048], w2_d[64:128, 1024:2048]).then_inc(WND, 16)
                scalar.dma_start(f6[64:128, 1:5:3, :], f6_d[64:128, 1:5:3, :]).then_inc(F14, 16)
                scalar.dma_start(f6[64:128, 0:6:5, :], f6_d[64:128, 0:6:5, :]).then_inc(F05, 16)
                scalar.wait_ge(SB, 1)
                # dummy first: walrus places the ACT table load before it
                aop(lambda: scalar.activation(dummy, dummy_in, AF.Exp))          # A1
                scalar.wait_ge(WND, 64)
                aop(lambda: scalar.activation(uuvv, w2, AF.Square, bias=1e-8))   # A2
                scalar.wait_ge(Vs, 1)
                aop(lambda: scalar.activation(lnr, r2, AF.Ln))                   # A3
                scalar.wait_ge(A, 3)  # ACT pipeline RAW on lnr
                aop(lambda: scalar.activation(ir2, lnr, AF.Exp, scale=-1.0))     # A4
                aop(lambda: scalar.activation(ir243, lnr, AF.Exp, scale=-1.0,
                                              bias=math.log(4.0 / 3.0)))         # A5

                srcmap = {"ss": (ss, 2), "cs": (cs, 3), "m12": (m12, 4), "m1m2": (m1m2, 5)}
                waited = [1]

                def exp_of(p):
                    srcname, sc, bi = espec[p]
                    src, need = srcmap[srcname]
                    if need > waited[0]:
                        scalar.wait_ge(Vs, need)
                        waited[0] = need
                    aop(lambda: scalar.activation(wts[p], src, AF.Exp, bias=bi, scale=sc))

                for p in pair_order[:5]:             # A6..A10 (w01..w1m1)
                    exp_of(p)
                aop(lambda: scalar.activation(q, ss, AF.Square, bias=-s8 / 2, scale=s8))   # A11
                exp_of((1, 2))                       # A12
                exp_of((1, -2))                      # A13
                scalar.wait_ge(A, 11)  # ACT pipeline RAW on q
                aop(lambda: scalar.activation(t8q, q, AF.Square, bias=-s2_, scale=s2_))    # A14
                exp_of((2, 0))                       # A15
                exp_of((2, -1))                      # A16
                scalar.wait_ge(A, 14)  # ACT pipeline RAW on t8q
                aop(lambda: scalar.activation(t8c2, t8q, AF.Identity, scale=C2))           # A17
                aop(lambda: scalar.activation(ser, q, AF.Identity, bias=C0 - C1 - C2, scale=C1))  # A18
                exp_of((2, 1))                       # A19
                exp_of((2, 2))                       # A20
                exp_of((2, -2))                      # A21
                scalar.wait_ge(FC, 16)
                aop(lambda: scalar.activation(flat3(f03), V(0, 0), AF.Copy, scale=0.3))    # A22
                assert a_count[0] == 22

            @block.vector
            def _(vector):
                # dx=0 pairsums as soon as fire center rows land
                vector.wait_ge(FC, 16)
                for p in [(0, 1), (0, 2)]:
                    vector.tensor_tensor(flat3(pst[p]), V(*p), V(-p[0], -p[1]), OP.add)
                vector.wait_ge(WND, 64)
                vector.tensor_tensor(uv, w2[:, 0:1024], w2[:, 1024:2048], OP.mult)
                vector.wait_ge(A, 2)
                vector.tensor_tensor(r2, uuvv[:, 0:1024], uuvv[:, 1024:2048], OP.add).then_inc(Vs, 1)
                # dx=1 pairsums while ACT does ln/exp
                vector.wait_ge(F14, 32)
                for p in [(1, 0), (1, 1), (1, -1), (1, 2), (1, -2)]:
                    vector.tensor_tensor(flat3(pst[p]), V(*p), V(-p[0], -p[1]), OP.add)
                vector.wait_ge(A, 4)
                vector.tensor_tensor(ss, uuvv[:, 1024:2048], ir2, OP.mult).then_inc(Vs, 1)
                vector.wait_ge(A, 5)
                vector.tensor_tensor(cs, uv, ir243, OP.mult).then_inc(Vs, 1)
                vector.tensor_tensor(m12, ss, cs, OP.add).then_inc(Vs, 1)
                vector.tensor_tensor(m1m2, ss, cs, OP.subtract).then_inc(Vs, 1)
                # MAC pairs 1-7 (dx<=1), then dx=2 pairsums, then pairs 8-12
                awaited = [5]

                def mac(i, p):
                    if exp_tick[p] > awaited[0]:
                        vector.wait_ge(A, exp_tick[p])
                        awaited[0] = exp_tick[p]
                    tgt = accv if i == 0 else prodv
                    vector.tensor_tensor(tgt, wts[p], pst[p], OP.mult)
                    if i > 0:
                        vector.tensor_tensor(accv, accv, prodv, OP.add)

                for i, p in enumerate(pair_order[:7]):
                    mac(i, p)
                vector.wait_ge(F05, 32)
                for p in pair_order[7:]:
                    vector.tensor_tensor(flat3(pst[p]), V(*p), V(-p[0], -p[1]), OP.add)
                for i, p in enumerate(pair_order[7:], start=7):
                    mac(i, p)
                vector.wait_ge(A, 18)
                vector.tensor_tensor(inv07, t8c2, ser, OP.add)
                vector.wait_ge(A, 22)
                for h in (0, 1):
                    hs = slice(h * 512, h * 512 + 512)
                    vector.tensor_tensor(flat3(spf, h), flat3(accv, h), V(0, 0, h), OP.add)
                    vector.tensor_tensor(sp07[:, hs], spf[:, hs], inv07[:, hs], OP.mult)
                    vector.tensor_tensor(opre[:, hs], sp07[:, hs], f03[:, hs], OP.add)
                    vector.tensor_scalar(
                        out=outt[:, hs], in0=opre[:, hs], scalar1=0.0, scalar2=1.0,
                        op0=OP.max, op1=OP.min,
                    ).then_inc(Vs, 1)   # Vs 6, 7

    return nc


def _get_nc():
    global _NC
    if _NC is None:
        _NC = _build_nc()
    return _NC


def _make_in_maps(fire_map, wind_u, wind_v):
    import ml_dtypes
    from numpy.lib.stride_tricks import sliding_window_view

    bf16 = ml_dtypes.bfloat16
    in_maps = []
    for b in range(B):
        fp = np.pad(
            np.asarray(fire_map[b, 0], np.float32), ((2, 2), (2, 2))
        ).astype(np.float16)
        for t in range(2):
            shard = fp[t * HS : t * HS + HS + 4]
            f6 = np.ascontiguousarray(
                sliding_window_view(shard, (6, 516))[::2, 0], dtype=np.float16
            )
            w2 = np.empty((128, 2048), bf16)
            w2[:, 0:1024] = np.asarray(
                wind_u[b, 0, t * HS : (t + 1) * HS], np.float32
            ).reshape(128, 1024).astype(bf16)
            w2[:, 1024:2048] = np.asarray(
                wind_v[b, 0, t * HS : (t + 1) * HS], np.float32
            ).reshape(128, 1024).astype(bf16)
            in_maps.append({"fire6": f6, "w2": w2})
    return in_maps


def _gather(results):
    out = np.empty((B, 1, H, W), np.float32)
    for ci, r in enumerate(results):
        b, t = divmod(ci, 2)
        out[b, 0, t * HS : (t + 1) * HS] = r["out"].astype(np.float32).reshape(HS, W)
    return out


def _run(fire_map, wind_u, wind_v, trace=False):
    from concourse.bass_utils import run_bass_kernel_spmd

    in_maps = _make_in_maps(fire_map, wind_u, wind_v)
    res = run_bass_kernel_spmd(_get_nc(), in_maps, list(range(N_CORES)), trace=trace)
    return _gather(res.results), res


def kernel(fire_map, wind_u, wind_v):
    out, _ = _run(fire_map, wind_u, wind_v, trace=False)
    return out


# revision 9
# speedup vs baseline: 1.9810x; 1.0662x over previous
"""Trainium2 Bass kernel for DirectionalConv2D (wind-directed 5x5 Gaussian blur).

Reference math (per pixel):
    theta = arctan2(v, u+1e-8);  c, s = cos(theta), sin(theta)
    w(dx,dy) = exp(-(dx*c + dy*s)^2 / 4.5)        for dx,dy in [-2..2]
    spread   = sum(w * fire[h+dx, w+dy]) / (sum(w) + 1e-8)   (zero padded)
    out      = clip(0.7*spread + 0.3*fire, 0, 1)

v4: three-way balance of DVE / ACT / DMA (rel-err budget 2e-2, sits ~6e-4):
  * ss = sin^2 = v^2/r2, cs = (4/3)uv/r2 via ir2 = Exp(-Ln(r2)), ir243 =
    Exp(-Ln(r2)+ln(4/3)); 12 symmetric pair weights = 12 Exp activations
    with args affine in ss/cs/m12/m1m2.  0.7/wsum is a cos(4k theta) series.
  * DVE 2x mode everywhere hot: wind path bf16, fire/weights/MAC fp16.
  * ACT does ONLY the chain (uu, vv, ln, 2 exps) + the 12 weight exps,
    dense: the series (q, t8q scale, ser) and 0.3*fire moved to cheap DVE
    TENSOR_SCALAR ops (single input stream, 2x) + one ACT Square (t8q).
  * Six late-consumed pairsums are built by the DMA engines instead of DVE:
    plain DRAM->SBUF copy of the +tap on the sync queue, then a gpsimd
    software-DGE DRAM->SBUF transfer of the -tap with accum_op=add.  Fire
    rows 0 and 5 are never loaded into SBUF (only those pairs used them).
  * uu/vv split so Square(u) starts when wu lands (wu ordered first on both
    HW queues); wind bf16 (512KB) and fire rows 1-4 (516KB) land ~13us.
  * Raw bass; monotone per-semaphore thresholds; finals + store in halves.
"""

import sys

if "/opt/trn_rl_repo" not in sys.path:
    sys.path.insert(0, "/opt/trn_rl_repo")

import numpy as np

B, H, W = 4, 512, 512
N_CORES = 8
HS = H // 2
KI = 1.0 / 4.5
C0 = 0.040093331769199714
C1 = 0.0007997721694363273
C2 = -1.6226127085146848e-06

_NC = None


def _build_nc():
    import math

    import concourse.bass as bass
    import concourse.mybir as mybir

    dt = mybir.dt
    AF = mybir.ActivationFunctionType
    OP = mybir.AluOpType
    k = KI
    f32 = dt.float32
    f16 = dt.float16
    bf16 = dt.bfloat16

    nc = bass.Bass(detect_race_conditions=False)

    f6_d = nc.dram_tensor("fire6", [128, 6, 516], f16, kind="ExternalInput")
    w2_d = nc.dram_tensor("w2", [128, 2048], bf16, kind="ExternalInput")  # wu|wv
    out_d = nc.dram_tensor("out", [128, 1024], f16, kind="ExternalOutput")

    def sb(name, shape, dtype=f32):
        return nc.alloc_sbuf_tensor(name, shape, dtype).ap()

    f6 = sb("f6", [128, 6, 516], f16)     # rows 0 and 5 never DMA'd (unused)
    w2 = sb("w2_t", [128, 2048], bf16)    # cols 0:1024 wu, 1024:2048 wv
    uuvv = sb("uuvv", [128, 2048], bf16)
    uv = sb("uv", [128, 1024], bf16)
    r2 = sb("r2", [128, 1024], bf16)
    lnr = sb("lnr", [128, 1024])
    ir2 = sb("ir2", [128, 1024], bf16)
    ir243 = sb("ir243", [128, 1024], bf16)
    ss = sb("ss", [128, 1024], f16)
    cs = sb("cs", [128, 1024], f16)       # holds (4/3)*sin*cos
    m12 = sb("m12", [128, 1024], f16)
    m1m2 = sb("m1m2", [128, 1024], f16)
    qy = sb("qy", [128, 1024], f16)
    q = sb("q", [128, 1024], f16)
    t8q = sb("t8q", [128, 1024], f16)
    t8c2 = sb("t8c2", [128, 1024], f16)
    ser = sb("ser", [128, 1024], f16)
    f03 = sb("f03", [128, 1024], f16)
    inv07 = sb("inv07", [128, 1024], f16)
    accv = sb("accv", [128, 1024], f16)
    prodv = sb("prodv", [128, 1024], f16)
    spf = sb("spf", [128, 1024], f16)
    sp07 = sb("sp07", [128, 1024], f16)
    opre = sb("opre", [128, 1024], f16)
    outt = sb("outt", [128, 1024], f16)
    dummy = sb("dummy_t", [128, 1])
    dummy_in = sb("dummy_in", [128, 1])

    # pairs 1-6 (DVE pairsums, fire rows 1-4 only), pairs 7-12 (DMA-built)
    pair_order = [
        (0, 1), (0, 2), (1, 0), (1, 1), (1, -1), (1, 2),
        (1, -2), (2, 0), (2, -1), (2, 1), (2, 2), (2, -2),
    ]
    dve_pairs = pair_order[:6]
    dma_pairs = pair_order[6:]
    wts = {p: sb(f"w{p[0]}_{p[1]}", [128, 1024], f16) for p in pair_order}
    pst = {p: sb(f"ps{p[0]}_{p[1]}", [128, 1024], f16) for p in pair_order}

    espec = {
        (0, 1): ("ss", -k, 0.0),
        (0, 2): ("ss", -4 * k, 0.0),
        (1, 0): ("ss", k, -k),
        (2, 0): ("ss", 4 * k, -4 * k),
        (1, 1): ("cs", -1.5 * k, -k),
        (1, -1): ("cs", 1.5 * k, -k),
        (1, 2): ("m12", -3 * k, -k),
        (1, -2): ("m1m2", -3 * k, -k),
        (2, -1): ("m12", 3 * k, -4 * k),
        (2, 1): ("m1m2", 3 * k, -4 * k),
        (2, 2): ("cs", -6 * k, -4 * k),
        (2, -2): ("cs", 6 * k, -4 * k),
    }

    def V(dx, dy, half=None):
        if half is None:
            return f6[:, 2 + dx : 4 + dx, 2 + dy : 514 + dy]
        return f6[:, 2 + dx + half, 2 + dy : 514 + dy]

    def VD(dx, dy):  # same tap window but in DRAM
        return f6_d[:, 2 + dx : 4 + dx, 2 + dy : 514 + dy]

    def flat3(ap, half=None):
        if half is None:
            return ap.rearrange("p (a b) -> p a b", a=2)
        return ap[:, half * 512 : half * 512 + 512]

    s8 = math.sqrt(8.0)
    s2_ = math.sqrt(2.0)
    bias_vals = sorted(
        {bi for _, _, bi in espec.values()} | {1e-8, -s2_, math.log(4.0 / 3.0)}
    )

    with (
        nc.semaphore("wu_s") as WU,
        nc.semaphore("wv_s") as WV,
        nc.semaphore("fc") as FC,
        nc.semaphore("f14") as F14,
        nc.semaphore("cps") as CPS,
        nc.semaphore("cpa") as CPA,
        nc.semaphore("ps") as PS,
        nc.semaphore("sqo") as SQO,
        nc.semaphore("sa") as A,
        nc.semaphore("sv") as Vs,
        nc.semaphore("sb") as SB,
    ):
        # ACT ticks: 1 dummy, 2 uu, 3 vv, 4 lnr, 5 ir2, 6 ir243,
        # 7..18 the 12 exps in pair_order, 19 t8q
        exp_tick = {p: 7 + i for i, p in enumerate(pair_order)}
        # DVE Vs ticks: 1 r2, 2 ss, 3 q, 4 cs, 5 m12, 6 m1m2, 7/8 out halves

        with nc.Block() as block:

            @block.sync
            def _(sync):
                sync.dma_start(w2[0:64, 0:1024], w2_d[0:64, 0:1024]).then_inc(WU, 16)
                sync.dma_start(w2[0:64, 1024:2048], w2_d[0:64, 1024:2048]).then_inc(WV, 16)
                sync.dma_start(f6[0:64, 1:5:3, :], f6_d[0:64, 1:5:3, :]).then_inc(F14, 16)
                # + taps of the DMA-built pairs (plain copies, HW queue)
                for p in dma_pairs[2:]:
                    sync.dma_start(flat3(pst[p]), VD(*p)).then_inc(CPS, 16)
                sync.wait_ge(Vs, 7)
                sync.dma_start(out_d[:, 0:512], outt[:, 0:512]).then_inc(SQO, 16)
                sync.wait_ge(Vs, 8)
                sync.dma_start(out_d[:, 512:1024], outt[:, 512:1024]).then_inc(SQO, 16)

            @block.gpsimd
            def _(gpsimd):
                gpsimd.dma_start(f6[:, 2:4, :], f6_d[:, 2:4, :]).then_inc(FC, 16)
                for bi_i, val in enumerate(bias_vals):
                    if (f32, val) in nc.const_aps.aps:
                        continue
                    t = nc.alloc_sbuf_tensor(f"constb{bi_i}", [128, 1], f32)
                    gpsimd.memset(t.ap(), val)
                    nc.const_aps.aps[(f32, val)] = t.ap()
                gpsimd.memset(dummy_in, 0.0).then_inc(SB, 1)
                # - taps accumulated over the copies (software-DGE compute)
                for i, p in enumerate(dma_pairs):
                    if i < 2:
                        gpsimd.wait_ge(CPA, 16 * (i + 1))
                    else:
                        gpsimd.wait_ge(CPS, 16 * (i - 1))
                    gpsimd.dma_start(
                        flat3(pst[p]), VD(-p[0], -p[1]), accum_op=OP.add
                    ).then_inc(PS, 16)

            @block.scalar
            def _(scalar):
                a_count = [0]

                def aop(emit):
                    emit().then_inc(A, 1)
                    a_count[0] += 1

                scalar.dma_start(w2[64:128, 0:1024], w2_d[64:128, 0:1024]).then_inc(WU, 16)
                scalar.dma_start(w2[64:128, 1024:2048], w2_d[64:128, 1024:2048]).then_inc(WV, 16)
                scalar.dma_start(f6[64:128, 1:5:3, :], f6_d[64:128, 1:5:3, :]).then_inc(F14, 16)
                for p in dma_pairs[:2]:
                    scalar.dma_start(flat3(pst[p]), VD(*p)).then_inc(CPA, 16)
                scalar.wait_ge(SB, 1)
                aop(lambda: scalar.activation(dummy, dummy_in, AF.Exp))               # A1
                scalar.wait_ge(WU, 32)
                aop(lambda: scalar.activation(uuvv[:, 0:1024], w2[:, 0:1024],
                                              AF.Square, bias=1e-8))                  # A2
                scalar.wait_ge(WV, 32)
                aop(lambda: scalar.activation(uuvv[:, 1024:2048], w2[:, 1024:2048],
                                              AF.Square, bias=1e-8))                  # A3
                scalar.wait_ge(Vs, 1)
                aop(lambda: scalar.activation(lnr, r2, AF.Ln))                        # A4
                scalar.wait_ge(A, 4)  # ACT pipeline RAW on lnr
                aop(lambda: scalar.activation(ir2, lnr, AF.Exp, scale=-1.0))          # A5
                aop(lambda: scalar.activation(ir243, lnr, AF.Exp, scale=-1.0,
                                              bias=math.log(4.0 / 3.0)))              # A6

                srcmap = {"ss": (ss, 2), "cs": (cs, 4), "m12": (m12, 5), "m1m2": (m1m2, 6)}
                waited = [1]
                for p in pair_order:                  # A7..A18, dense
                    srcname, sc, bi = espec[p]
                    src, need = srcmap[srcname]
                    if need > waited[0]:
                        scalar.wait_ge(Vs, need)
                        waited[0] = need
                    aop(lambda src=src, sc=sc, bi=bi, p=p:
                        scalar.activation(wts[p], src, AF.Exp, bias=bi, scale=sc))
                aop(lambda: scalar.activation(t8q, q, AF.Square, bias=-s2_, scale=s2_))  # A19
                assert a_count[0] == 19

            @block.vector
            def _(vector):
                vector.wait_ge(FC, 16)
                for p in [(0, 1), (0, 2)]:
                    vector.tensor_tensor(flat3(pst[p]), V(*p), V(-p[0], -p[1]), OP.add)
                vector.tensor_scalar_mul(flat3(f03), V(0, 0), 0.3)
                vector.wait_ge(WV, 32)
                vector.tensor_tensor(uv, w2[:, 0:1024], w2[:, 1024:2048], OP.mult)
                vector.wait_ge(A, 3)
                vector.tensor_tensor(r2, uuvv[:, 0:1024], uuvv[:, 1024:2048], OP.add).then_inc(Vs, 1)
                vector.wait_ge(F14, 32)
                for p in [(1, 0), (1, 1), (1, -1), (1, 2)]:
                    vector.tensor_tensor(flat3(pst[p]), V(*p), V(-p[0], -p[1]), OP.add)
                vector.wait_ge(A, 5)
                vector.tensor_tensor(ss, uuvv[:, 1024:2048], ir2, OP.mult).then_inc(Vs, 1)
                vector.tensor_scalar(out=qy, in0=ss, scalar1=s8, scalar2=-s8 / 2,
                                     op0=OP.mult, op1=OP.add)
                vector.tensor_tensor(q, qy, qy, OP.mult).then_inc(Vs, 1)
                vector.wait_ge(A, 6)
                vector.tensor_tensor(cs, uv, ir243, OP.mult).then_inc(Vs, 1)
                vector.tensor_tensor(m12, ss, cs, OP.add).then_inc(Vs, 1)
                vector.tensor_tensor(m1m2, ss, cs, OP.subtract).then_inc(Vs, 1)
                # MAC
                awaited = [6]
                ps_waited = [0]
                for i, p in enumerate(pair_order):
                    if exp_tick[p] > awaited[0]:
                        vector.wait_ge(A, exp_tick[p])
                        awaited[0] = exp_tick[p]
                    if i >= 6:
                        need = 16 * (i - 5)
                        if need > ps_waited[0]:
                            vector.wait_ge(PS, need)
                            ps_waited[0] = need
                    tgt = accv if i == 0 else prodv
                    vector.tensor_tensor(tgt, wts[p], pst[p], OP.mult)
                    if i > 0:
                        vector.tensor_tensor(accv, accv, prodv, OP.add)
                # series tail + finals
                vector.wait_ge(A, 19)
                vector.tensor_scalar_mul(t8c2, t8q, C2)
                vector.tensor_scalar(out=ser, in0=q, scalar1=C1, scalar2=C0 - C1 - C2,
                                     op0=OP.mult, op1=OP.add)
                vector.tensor_tensor(inv07, t8c2, ser, OP.add)
                for h in (0, 1):
                    hs = slice(h * 512, h * 512 + 512)
                    vector.tensor_tensor(flat3(spf, h), flat3(accv, h), V(0, 0, h), OP.add)
                    vector.tensor_tensor(sp07[:, hs], spf[:, hs], inv07[:, hs], OP.mult)
                    vector.tensor_tensor(opre[:, hs], sp07[:, hs], f03[:, hs], OP.add)
                    vector.tensor_scalar(
                        out=outt[:, hs], in0=opre[:, hs], scalar1=0.0, scalar2=1.0,
                        op0=OP.max, op1=OP.min,
                    ).then_inc(Vs, 1)   # Vs 7, 8

    return nc


def _get_nc():
    global _NC
    if _NC is None:
        _NC = _build_nc()
    return _NC


def _make_in_maps(fire_map, wind_u, wind_v):
    import ml_dtypes
    from numpy.lib.stride_tricks import sliding_window_view

    bf16 = ml_dtypes.bfloat16
    in_maps = []
    for b in range(B):
        fp = np.pad(
            np.asarray(fire_map[b, 0], np.float32), ((2, 2), (2, 2))
        ).astype(np.float16)
        for t in range(2):
            shard = fp[t * HS : t * HS + HS + 4]
            f6 = np.ascontiguousarray(
                sliding_window_view(shard, (6, 516))[::2, 0], dtype=np.float16
            )
            w2 = np.empty((128, 2048), bf16)
            w2[:, 0:1024] = np.asarray(
                wind_u[b, 0, t * HS : (t + 1) * HS], np.float32
            ).reshape(128, 1024).astype(bf16)
            w2[:, 1024:2048] = np.asarray(
                wind_v[b, 0, t * HS : (t + 1) * HS], np.float32
            ).reshape(128, 1024).astype(bf16)
            in_maps.append({"fire6": f6, "w2": w2})
    return in_maps


def _gather(results):
    out = np.empty((B, 1, H, W), np.float32)
    for ci, r in enumerate(results):
        b, t = divmod(ci, 2)
        out[b, 0, t * HS : (t + 1) * HS] = r["out"].astype(np.float32).reshape(HS, W)
    return out


def _run(fire_map, wind_u, wind_v, trace=False):
    from concourse.bass_utils import run_bass_kernel_spmd

    in_maps = _make_in_maps(fire_map, wind_u, wind_v)
    res = run_bass_kernel_spmd(_get_nc(), in_maps, list(range(N_CORES)), trace=trace)
    return _gather(res.results), res


def kernel(fire_map, wind_u, wind_v):
    out, _ = _run(fire_map, wind_u, wind_v, trace=False)
    return out
